# revision 7
# baseline (speedup 1.0000x reference)
# Multi-head self-attention (b=2, s=2048, d=1024, 16 heads) on 8 Trainium2
# NeuronCores, head-sharded (2 heads per core).
#
# Layout strategy (per core):
#   - Everything is computed in the "transposed" [j, i] layout so that:
#       * the key-mask becomes a per-partition activation bias (free),
#       * the context matmul consumes attn^T and natural-layout V directly,
#       * softmax sums come for free from a ones-column appended to V.
#   - scoresT[j, i] = K^T(d,j)^T-packed matmul; 2 heads are row-packed into
#     the 128x128 PE array (d=64 each) and run concurrently.
#   - exp via ScalarE activation: Exp(SCALE*x + mask_bias[j]).  Softmax max-
#     subtraction is skipped: |SCALE*scores| < 1e-10 for any realistic input
#     (SCALE = 1024^-5), so exp can never overflow and the result matches the
#     max-subtracted softmax bit-for-bit in fp32.
#   - ctx~[d+1, i] = sum_j v'[j, d+1] * expT[j, i] accumulated over j-tiles;
#     row d is the ones row -> softmax denominator.
#   - normalization: recip = 1/sums broadcast across partitions with a K=1
#     ones matmul; attnT tiles normalized in-place on VectorE and DMAd out.
#   - o-proj: outT[f, i] = Wo_slice^T-packed matmul over the core's 128 ctx
#     dims; partial outputs summed across cores on the host.
#
# The host shards heads, pre-transposes/casts x once, and un-shards by
# transposing attnT -> attn and summing the 8 o-proj partials.

import os
import numpy as np
import ml_dtypes

B, S, D = 2, 2048, 1024
N_HEADS = 16
HD = 64                    # head dim
NCORES = 8
HL = N_HEADS // NCORES     # heads per core (2)
M = HL * HD                # head-dims per core (128)
P = 128
SCALE = float(D ** -5)
NEG = float(np.float32(-3.4028234663852886e38))

_CACHE = {}
LAST = {}


def _build_nc(s=S, b=B, d=D, reps=1):
    import concourse.mybir as mybir
    import concourse.tile as tile
    from concourse import bacc

    dt = mybir.dt
    T = b * s              # total tokens
    NB = T // P            # token tiles
    KT = d // P            # contraction tiles for projections
    SJT = s // P           # j-tiles per batch
    IC = s // 512          # 512-wide i chunks per batch

    nc = bacc.Bacc("TRN2", target_bir_lowering=False, debug=False)

    xT = nc.dram_tensor("xT", [d, T], dt.bfloat16, kind="ExternalInput").ap()
    wqT = nc.dram_tensor("wqT", [d, M], dt.bfloat16, kind="ExternalInput").ap()
    wkT = nc.dram_tensor("wkT", [d, M], dt.bfloat16, kind="ExternalInput").ap()
    wvT = nc.dram_tensor("wvT", [d, M], dt.bfloat16, kind="ExternalInput").ap()
    woT = nc.dram_tensor("woT", [M, d], dt.bfloat16, kind="ExternalInput").ap()
    mb = nc.dram_tensor("mb", [P, b * SJT], dt.float32, kind="ExternalInput").ap()
    attnT = nc.dram_tensor(
        "attnT", [HL, b, s, s], dt.bfloat16, kind="ExternalOutput"
    ).ap()
    outT = nc.dram_tensor("outT", [d, T], dt.float32, kind="ExternalOutput").ap()

    with tile.TileContext(nc) as tc:
        with (
            tc.tile_pool(name="persist", bufs=1) as pp,
            tc.tile_pool(name="stage", bufs=3) as sp,
        ):
            qT = pp.tile([P, T], dt.bfloat16, name="qT")
            kT = pp.tile([P, T], dt.bfloat16, name="kT")
            vv = pp.tile([P, NB, 2 * (HD + 1)], dt.bfloat16, name="vv")
            rb_sb = [pp.tile([P, s], dt.bfloat16, name=f"rb{h}") for h in range(HL)]
            ctxT = pp.tile([P, T], dt.bfloat16, name="ctxT")
            mb_sb = pp.tile([P, b * SJT], dt.float32, name="mbs")
            ones_sb = pp.tile([1, P], dt.float32, name="ones")
            wo_sb = pp.tile([P, d], dt.bfloat16, name="wos")

            nc.vector.memset(ones_sb[:], 1.0)
            nc.vector.memset(vv[:, :, HD : HD + 1], 1.0)
            nc.vector.memset(vv[:, :, 2 * HD + 1 : 2 * HD + 2], 1.0)
            nc.sync.dma_start(out=mb_sb[:], in_=mb[:, :])
            nc.sync.dma_start(out=wo_sb[:], in_=woT[:, :])

            for _rep in range(reps):
                _run_body(nc, tc, mybir, dt, sp, s, b, d, qT, kT, vv, rb_sb, ctxT,
                          mb_sb, ones_sb, wo_sb, xT, wqT, wkT, wvT, attnT, outT)

    nc.compile()
    return nc


def _run_body(nc, tc, mybir, dt, sp, s, b, d, qT, kT, vv, rb_sb, ctxT,
              mb_sb, ones_sb, wo_sb, xT, wqT, wkT, wvT, attnT, outT):
    T = b * s
    NB = T // P
    KT = d // P
    SJT = s // P
    IC = s // 512
    if True:
        if True:
            # ---------------- phase 1: q/k/v projections ----------------
            with (
                tc.tile_pool(name="xw", bufs=1) as xp,
                tc.tile_pool(name="qkps", bufs=2, space="PSUM") as qps,
                tc.tile_pool(name="vps", bufs=2, space="PSUM") as vps,
            ):
                xt = xp.tile([P, KT, T], dt.bfloat16, name="xt")
                wq = xp.tile([P, KT, M], dt.bfloat16, name="wq")
                wk = xp.tile([P, KT, M], dt.bfloat16, name="wk")
                wv = xp.tile([P, KT, M], dt.bfloat16, name="wv")
                for kt in range(KT):
                    nc.sync.dma_start(
                        out=xt[:, kt, :], in_=xT[kt * P : (kt + 1) * P, :]
                    )
                nc.sync.dma_start(out=wq[:], in_=wqT.rearrange("(k p) m -> p k m", p=P))
                nc.sync.dma_start(out=wk[:], in_=wkT.rearrange("(k p) m -> p k m", p=P))
                nc.sync.dma_start(out=wv[:], in_=wvT.rearrange("(k p) m -> p k m", p=P))

                # q, k in [m, i] layout (m = 2*64 packed head dims)
                for wsb, dest in ((wq, qT), (wk, kT)):
                    for ich in range(T // 512):
                        ps = qps.tile([P, 512], dt.float32, tag="qk")
                        for kt in range(KT):
                            nc.tensor.matmul(
                                ps[:],
                                wsb[:, kt, :],
                                xt[:, kt, ich * 512 : (ich + 1) * 512],
                                start=(kt == 0),
                                stop=(kt == KT - 1),
                            )
                        nc.vector.tensor_copy(
                            dest[:, ich * 512 : (ich + 1) * 512], ps[:]
                        )
                # v in natural [token, d'] layout with ones columns
                for it in range(NB):
                    ps = vps.tile([P, M], dt.float32, tag="v")
                    for kt in range(KT):
                        nc.tensor.matmul(
                            ps[:],
                            xt[:, kt, it * P : (it + 1) * P],
                            wv[:, kt, :],
                            start=(kt == 0),
                            stop=(kt == KT - 1),
                        )
                    nc.vector.tensor_copy(vv[:, it, 0:HD], ps[:, 0:HD])
                    nc.vector.tensor_copy(vv[:, it, HD + 1 : 2 * HD + 1], ps[:, HD:M])

            # ---------------- phase 2: attention per batch ----------------
            with (
                tc.tile_pool(name="exp", bufs=HL * SJT) as ep,
                tc.tile_pool(name="scps", bufs=2, space="PSUM") as scps,
                tc.tile_pool(name="ctps", bufs=2, space="PSUM") as ctps,
                tc.tile_pool(name="rbps", bufs=2, space="PSUM") as rbps,
            ):
                for bb in range(b):
                    expt = [
                        [
                            ep.tile(
                                [P, s], dt.bfloat16, tag="exp", name=f"exp{h}_{jt}"
                            )
                            for jt in range(SJT)
                        ]
                        for h in range(HL)
                    ]
                    # scoresT + exp; 2 heads row-packed in the PE array
                    for jt in range(SJT):
                        for h in range(HL):
                            for ih in range(s // 1024):
                                ps = scps.tile([P, 1024], dt.float32, tag="sc")
                                for n in range(2):
                                    i0 = bb * s + ih * 1024 + n * 512
                                    nc.tensor.matmul(
                                        ps[:, n * 512 : (n + 1) * 512],
                                        kT[
                                            h * HD : (h + 1) * HD,
                                            bb * s + jt * P : bb * s + (jt + 1) * P,
                                        ],
                                        qT[h * HD : (h + 1) * HD, i0 : i0 + 512],
                                        start=True,
                                        stop=True,
                                    )
                                nc.scalar.activation(
                                    expt[h][jt][:, ih * 1024 : (ih + 1) * 1024],
                                    ps[:],
                                    mybir.ActivationFunctionType.Exp,
                                    bias=mb_sb[:, bb * SJT + jt : bb * SJT + jt + 1],
                                    scale=SCALE,
                                )
                    # context accumulation + normalization per head
                    for h in range(HL):
                        for ic in range(IC):
                            cps = ctps.tile([HD + 1, 512], dt.float32, tag="ctx")
                            for jt in range(SJT):
                                nc.tensor.matmul(
                                    cps[:],
                                    vv[
                                        :,
                                        bb * SJT + jt,
                                        h * (HD + 1) : (h + 1) * (HD + 1),
                                    ],
                                    expt[h][jt][:, ic * 512 : (ic + 1) * 512],
                                    start=(jt == 0),
                                    stop=(jt == SJT - 1),
                                )
                            rsb = sp.tile([1, 512], dt.float32, tag="recip")
                            nc.vector.reciprocal(rsb[:], cps[HD : HD + 1, :])
                            rps = rbps.tile([P, 512], dt.float32, tag="rb")
                            nc.tensor.matmul(
                                rps[:], ones_sb[:], rsb[:], start=True, stop=True
                            )
                            rb32 = sp.tile([P, 512], dt.float32, tag="rb32")
                            nc.vector.tensor_copy(rb32[:], rps[:])
                            nc.vector.tensor_mul(
                                ctxT[
                                    h * HD : (h + 1) * HD,
                                    bb * s + ic * 512 : bb * s + (ic + 1) * 512,
                                ],
                                cps[0:HD, :],
                                rb32[0:HD, :],
                            )
                            nc.vector.tensor_copy(
                                rb_sb[h][:, ic * 512 : (ic + 1) * 512], rps[:]
                            )
                        for jt in range(SJT):
                            nc.vector.tensor_mul(
                                expt[h][jt][:], expt[h][jt][:], rb_sb[h][:]
                            )
                            nc.sync.dma_start(
                                out=attnT[h, bb, jt * P : (jt + 1) * P, :],
                                in_=expt[h][jt][:],
                            )

            # ---------------- phase 3: output projection ----------------
            with tc.tile_pool(name="ops", bufs=4, space="PSUM") as ops:
                for ft in range(d // P):
                    for nch in range(T // 1024):
                        ost = sp.tile([P, 1024], dt.float32, tag="ostage")
                        for n in range(2):
                            ps = ops.tile([P, 512], dt.float32, tag="op")
                            nc.tensor.matmul(
                                ps[:],
                                wo_sb[:, ft * P : (ft + 1) * P],
                                ctxT[:, nch * 1024 + n * 512 : nch * 1024 + (n + 1) * 512],
                                start=True,
                                stop=True,
                            )
                            nc.vector.tensor_copy(ost[:, n * 512 : (n + 1) * 512], ps[:])
                        nc.sync.dma_start(
                            out=outT[
                                ft * P : (ft + 1) * P, nch * 1024 : (nch + 1) * 1024
                            ],
                            in_=ost[:],
                        )


def _get_nc():
    if "nc" not in _CACHE:
        _CACHE["nc"] = _build_nc()
    return _CACHE["nc"]


def _bf16_to_f32(a):
    # exact bf16 -> fp32 widening via bit shift (fast path for big arrays)
    u = np.ascontiguousarray(a).view(np.uint16).astype(np.uint32) << 16
    return u.view(np.float32)


def kernel(x, mask, Wq, Wk, Wv, Wo):
    bf = ml_dtypes.bfloat16
    x = np.asarray(x, dtype=np.float32)
    mask = np.asarray(mask)
    Wq = np.asarray(Wq, dtype=np.float32)
    Wk = np.asarray(Wk, dtype=np.float32)
    Wv = np.asarray(Wv, dtype=np.float32)
    Wo = np.asarray(Wo, dtype=np.float32)

    xT = np.ascontiguousarray(x.reshape(B * S, D).T).astype(bf)  # [D, B*S]
    mbias = np.where(mask.astype(bool), np.float32(0.0), np.float32(NEG))
    mbias = np.ascontiguousarray(
        mbias.reshape(B, S // P, P).transpose(2, 0, 1).reshape(P, B * (S // P))
    ).astype(np.float32)

    in_maps = []
    for c in range(NCORES):
        sl = slice(c * M, (c + 1) * M)
        in_maps.append(
            {
                "xT": xT,
                "mb": mbias,
                "wqT": np.ascontiguousarray(Wq[sl, :].T).astype(bf),
                "wkT": np.ascontiguousarray(Wk[sl, :].T).astype(bf),
                "wvT": np.ascontiguousarray(Wv[sl, :].T).astype(bf),
                "woT": np.ascontiguousarray(Wo[:, sl].T).astype(bf),
            }
        )

    from concourse.bass_utils import run_bass_kernel_spmd

    nc = _get_nc()
    LAST["in_maps"] = in_maps
    res = run_bass_kernel_spmd(nc, in_maps, core_ids=list(range(NCORES)))
    LAST["exec_time_ns"] = res.exec_time_ns
    LAST["mean_exec_time_ns"] = res.mean_exec_time_ns
    LAST["results"] = res

    attn = np.empty((N_HEADS, B, S, S), np.float32)
    out = np.zeros((B * S, D), np.float32)
    for c in range(NCORES):
        r = res.results[c]
        at = r["attnT"]  # [HL, B, S(j), S(i)] bf16
        at32 = _bf16_to_f32(at)
        for hl in range(HL):
            for bb in range(B):
                attn[c * HL + hl, bb] = at32[hl, bb].T
        out += r["outT"].T
    return out.reshape(B, S, D), attn
